# revision 3
# baseline (speedup 1.0000x reference)
"""MinLSTM fused kernel for TRN2 (8 NeuronCores, one batch element each).

Math (equivalent to the reference's log-space form):
    zf = x@Wf+bf ; zi = x@Wi+bi ; zh = x@Wh+bh
    Ef = exp(-zf); ti = tanh((zi+bi)/2); th = tanh((zh+bh)/2)
    si = (1+ti)/2 ; sh = (1+th)/2                  # sigmoid via tanh
    g  = max(zh + bh + 0.5, sh)
    p~ = (1+Ef)*(1+ti)        # = 2p,  p = si/sf
    u~ = p~*g                 # = 2u
    S~ = 1 + cumsum(u~)       # = 2S (fp32 scan state)
    r2~ = 1/(2+p~)            # = r2/2
    out[:,0,:] = 0.5 ; out[:,t+1,:] = S~[t]*r2~[t]   # = S*r2 (host-side)

Schedule (per core, z^T = [512h, 4096t], 4 h-chunks of 128):
  - Per h: zf -> Ef(exp), zi -> ti(tanh), zh -> th(tanh). tanh shares the
    exp ACT table set, so the only table switches are the two recip phases
    (4 loads total).  PSUM units [128, 2048] (4 banks, 2 in flight), MM
    order d-outer/tc-inner; bass emits one LDWEIGHTS per matmul, so
    _dedup_ldweights() strips consecutive same-weight reloads from the BIR.
  - h-gate GEMM in fp8e4 DoubleRow (W*8, x/8 host-quantized, net scale 1;
    f/i gates stay bf16 -- fp8 there breaks the 2e-2 gate).
  - Per h the gate order is zi -> zf -> zh (tanh and exp share a table,
    so ACT follows data order), which feeds the DVE ~8us earlier.
  - DVE: ip1/sh tensor_scalars (4x mode), g/p~ STTs, u~ TT (2x),
    pair-scan (scan consumes u~ even via data0 and odd via data1 -- halves
    the serial scan steps); even positions fixed up as S[2k]=S[2k-1]+u[2k]
    (gpsimd for h0..2, DVE for h3; u~ stays on DVE -- gpsimd writes to the
    scan input contend with the scan's SBUF reads).  h3 runs its chain at
    quarter width to shorten the kernel tail.
  - S~ and r2~ DMA out bf16; host computes S~*r2~, transposes, prepends
    the t=0 column (0.5).
"""
import os
import numpy as np

_CACHE = {}

B, T, D, H = 8, 4096, 512, 512
NCORES = 8
N_HC = H // 128       # 4 h-chunks
N_D = D // 128        # 4 contraction chunks
HALF = 2048           # PSUM unit width (4 banks)
MMW = 512             # cols per matmul (1 bank)

FP8_H = os.environ.get("K2_FP8H", "1") == "1"      # h-gate in fp8 DR
LDW_OPT = os.environ.get("K2_LDWOPT", "1") == "1"  # BIR-level dedup  # walrus ldweights dedup
U_ON_GP = os.environ.get("K2_UGP", "0") == "1"     # u on DVE: avoids gp/DVE SBUF contention on scans
XSCALE = 8.0                                        # W*8, x/8 for fp8


def _install_tilefix():
    """This walrus build accepts only ONE sync wait per hardware instruction;
    Tile can emit several. Spill extras onto injected single-wait drains."""
    import concourse.tile as tile
    from concourse import mybir
    from concourse.vector_clock import ScopedClock

    if getattr(tile.TileContext, "_minlstm_patched", False):
        return
    orig_lower = tile.TileContext._lower_ordered_insts

    def _spill_waits(self, ordered):
        nc = self.nc
        for bb_name, insts in ordered.items():
            out = []
            for inst in insts:
                si = inst.sync_info
                if si is not None and len(si.on_wait) > 1 and inst.engine is not None:
                    waits = list(si.on_wait)
                    for w in waits[:-1]:
                        d = mybir.InstDrain(
                            name=nc.get_next_instruction_name(),
                            ins=[], outs=[], bass_is_fusable=False,
                            sync_info=mybir.SyncInfo(on_wait=[w], on_update=[]),
                        )
                        d.engine = inst.engine
                        out.append(d)
                    si.on_wait = [waits[-1]]
                out.append(inst)
            insts[:] = out
        return ordered

    def _patched_lower(self, ordered):
        return orig_lower(self, _spill_waits(self, ordered))

    def _split_drain_and_barrier(self, tick_clock, wait_clock):
        drain_inst = self.nc.sync.drain()
        wait_clock.add_sem_waits(
            drain_inst.ins, ScopedClock({None: tick_clock.global_clock})
        )
        si = drain_inst.ins.sync_info
        if si is not None and len(si.on_wait) > 1:
            waits = list(si.on_wait)
            si.on_wait = [waits[0]]
            for w in waits[1:]:
                extra = self.nc.sync.drain()
                esi = extra.ins.sync_info
                if esi is None:
                    extra.ins.sync_info = mybir.SyncInfo(on_wait=[w], on_update=[])
                else:
                    esi.on_wait = [w]
        self.nc.all_engine_barrier()
        assert self.sems is not None
        popped = self.nc._tile_sem_poison_stack.pop()
        assert popped is self._sem_poison
        self.nc.clear_and_free_semaphores(list(self.sems.allocated().values()))
        self.nc.all_engine_barrier()

    tile.TileContext._lower_ordered_insts = _patched_lower
    tile.TileContext._drain_and_barrier = _split_drain_and_barrier
    tile.TileContext._minlstm_patched = True


def _dedup_ldweights(bir_bytes):
    """Drop PE Ldweights whose stationary AP equals the previous load and
    which carry no semaphore waits/updates (bass legalization emits one per
    matmul; consecutive same-weight matmuls don't need the reload)."""
    import json as _json

    d = _json.loads(bir_bytes)
    for fn in d["functions"]:
        for blk in fn["blocks"]:
            out = []
            prev_key = None
            for inst in blk["instructions"]:
                if inst.get("engine") == "PE" and inst.get("opcode") == "Ldweights":
                    key = _json.dumps(
                        [inst.get("ins"), inst.get("perf_mode"),
                         inst.get("tile_position"), inst.get("tile_size"),
                         inst.get("is_transpose")], sort_keys=True)
                    si = inst.get("sync_info")
                    clean = not si or (not si.get("on_wait")
                                       and not si.get("on_update"))
                    if key == prev_key and clean:
                        continue
                    prev_key = key
                out.append(inst)
            blk["instructions"] = out
    return _json.dumps(d).encode()


def _build():
    import concourse.bass as bass
    import concourse.tile as tile
    from concourse import mybir
    from concourse.tile_rust import add_dep_helper

    _install_tilefix()

    f32 = mybir.dt.float32
    bf16 = mybir.dt.bfloat16
    fp8 = mybir.dt.float8e4
    AF = mybir.ActivationFunctionType
    ALU = mybir.AluOpType
    PM = mybir.MatmulPerfMode

    nc = bass.Bass("TRN2", target_bir_lowering=False, debug=False,
                   num_devices=NCORES)

    # host layout:
    #   wt  [512, 1024]  bf16: per 128-row d-block [Wf_d | Wi_d]
    #   xt  [512, 4096]  bf16: x^T
    #   x8  [256, 8192]  fp8 : kpair-tiles [128, 2, 4096] stacked
    #   w8  [256, 1024]  fp8 : kpair-tiles [128, 2, 512]  stacked
    #   bt  [128, 16]    f32 : [-bf | bi | bh+0.5 | bh] per h-chunk col
    wt_d = nc.dram_tensor("wt", [D, 2 * H], bf16, kind="ExternalInput").ap()
    xt_d = nc.dram_tensor("xt", [D, T], bf16, kind="ExternalInput").ap()
    x8_d = nc.dram_tensor("x8", [2 * 128, 2 * T], fp8, kind="ExternalInput").ap()
    w8_d = nc.dram_tensor("w8", [2 * 128, 2 * H], fp8, kind="ExternalInput").ap()
    bias_d = nc.dram_tensor("biases", [128, 16], f32, kind="ExternalInput").ap()
    S_d = nc.dram_tensor("S", [H, T], bf16, kind="ExternalOutput").ap()
    r2_d = nc.dram_tensor("r2", [H, T], bf16, kind="ExternalOutput").ap()

    prev_act = [None]

    def act_raw(out, in_, func, bias=0.0, scale=1.0):
        eng = nc.scalar
        inputs = [eng.lower_ap(in_)]
        for arg in (bias, scale, 0.0):
            if isinstance(arg, bass.AP):
                inputs.append(eng.lower_ap(arg))
            else:
                inputs.append(
                    mybir.ImmediateValue(dtype=f32, value=float(arg))
                )
        i = eng.add_instruction(
            mybir.InstActivation(
                name=nc.get_next_instruction_name(),
                func=func, ins=inputs, outs=[eng.lower_ap(out)],
            )
        )
        if prev_act[0] is not None:
            add_dep_helper(i.ins, prev_act[0].ins, sync=False,
                           reason="ACT table-set order")
        prev_act[0] = i
        return i

    with tile.TileContext(nc) as tc:
        with (
            tc.tile_pool(name="wp", bufs=1) as wp,
            tc.tile_pool(name="xp", bufs=1) as xp,
            tc.tile_pool(name="cons", bufs=1) as cons,
            tc.tile_pool(name="ps", bufs=2, space="PSUM") as ps,
            tc.tile_pool(name="Efp", bufs=2) as Efp,
            tc.tile_pool(name="shp", bufs=2) as shp,
            tc.tile_pool(name="gp_", bufs=2) as gp_,
            tc.tile_pool(name="pp", bufs=4) as pp,
            tc.tile_pool(name="up", bufs=2) as up,
            tc.tile_pool(name="Sp", bufs=2) as Sp,
            tc.tile_pool(name="rp", bufs=2) as rp,
        ):
            wt = [wp.tile([128, 2 * H], bf16, tag=f"wt{d}", name=f"wt{d}")
                  for d in range(N_D)]
            xt = [xp.tile([128, T], bf16, tag=f"xt{d}", name=f"xt{d}")
                  for d in range(N_D)]
            x8t = [xp.tile([128, 2, T], fp8, tag=f"x8_{k}", name=f"x8_{k}")
                   for k in range(2)]
            w8t = [wp.tile([128, 2, H], fp8, tag=f"w8_{k}", name=f"w8_{k}")
                   for k in range(2)]
            # DMA order = consumption order: W, xT half0, xT half1, x8, w8
            for d in range(N_D):
                nc.sync.dma_start(wt[d][:], wt_d[128 * d:128 * (d + 1), :])
            bt = cons.tile([128, 16], f32, tag="bt")
            nc.sync.dma_start(bt[:], bias_d[:])
            for d in range(N_D):
                nc.sync.dma_start(xt[d][:, 0:HALF],
                                  xt_d[128 * d:128 * (d + 1), 0:HALF])
            for d in range(N_D):
                nc.sync.dma_start(xt[d][:, HALF:T],
                                  xt_d[128 * d:128 * (d + 1), HALF:T])
            for k in range(2):
                nc.sync.dma_start(w8t[k][:], w8_d[128 * k:128 * (k + 1), :])
                nc.sync.dma_start(x8t[k][:], x8_d[128 * k:128 * (k + 1), :])

            zero1 = cons.tile([128, 8], f32, tag="zero1")
            nc.vector.memset(zero1[:], 0.0)
            zb16 = cons.tile([128, HALF], bf16, tag="zb16")
            nc.vector.memset(zb16[:], 0.0)
            junk = cons.tile([128, 640], bf16, tag="junk")
            nc.gpsimd.memset(junk[:], 0.0)
            # warm up the PE p-state while the input DMAs land
            for i in range(12):
                wz = ps.tile([128, HALF], f32, tag="z", name=f"warm{i}")
                nc.tensor.matmul(wz[:, 0:512], junk[:, 0:128], junk[:, 128:640],
                                 start=True, stop=True)

            def gemm_bf16(gate, h, half, name):
                """z^T [128h, 2048t] for gate 0(f)/1(i), bf16 weights."""
                z = ps.tile([128, HALF], f32, tag="z", name=name)
                wc = 512 * gate + 128 * h
                t0 = HALF * half
                for d in range(N_D):
                    for tc_ in range(HALF // MMW):
                        nc.tensor.matmul(
                            z[:, MMW * tc_:MMW * (tc_ + 1)],
                            wt[d][:, wc:wc + 128],
                            xt[d][:, t0 + MMW * tc_:t0 + MMW * (tc_ + 1)],
                            start=(d == 0), stop=(d == N_D - 1),
                        )
                return z

            def gemm_fp8(h, half, name):
                """zh^T [128h, 2048t] via fp8 DoubleRow (2 kpairs)."""
                z = ps.tile([128, HALF], f32, tag="z", name=name)
                t0 = HALF * half
                for k in range(2):
                    for tc_ in range(HALF // MMW):
                        nc.tensor.matmul(
                            z[:, MMW * tc_:MMW * (tc_ + 1)],
                            w8t[k][:, :, 128 * h:128 * (h + 1)],
                            x8t[k][:, :, t0 + MMW * tc_:t0 + MMW * (tc_ + 1)],
                            start=(k == 0), stop=(k == 1),
                            perf_mode=PM.DoubleRow,
                        )
                return z

            def gemm_h(h, half, name):
                if FP8_H:
                    return gemm_fp8(h, half, name)
                return gemm_bf16(2, h, half, name)  # unused unless FP8_H=0

            def phase_IH(h):
                """Gate order zi -> zf -> zh (one ACT table: tanh+exp).
                ip1/sh TS, p~/g STT, u~ = p~*g, pair-scan (S~ = 2S,
                init 1.0), even-fixups."""
                nbf_ap = bt[:, h:h + 1]
                bi2_ap = bt[:, 4 + h:5 + h]
                bg_ap = bt[:, 8 + h:9 + h]
                bh2_ap = bt[:, 12 + h:13 + h]
                g = gp_.tile([128, T], bf16, tag="g", name=f"g{h}")
                p = pp.tile([128, T], bf16, tag="p", name=f"p{h}")
                last = h == N_HC - 1
                ip1s = []
                for half in range(2):
                    zi = gemm_bf16(1, h, half, f"zi{h}_{half}")
                    ti = shp.tile([128, HALF], bf16, tag="ti",
                                  name=f"ti{h}_{half}")
                    act_raw(ti[:], zi[:], AF.Tanh, bias=bi2_ap, scale=0.5)
                    ip1 = shp.tile([128, HALF], bf16, tag="ip1",
                                   name=f"ip1_{h}_{half}")
                    nc.vector.tensor_scalar(out=ip1[:], in0=ti[:],
                                            scalar1=1.0, scalar2=None,
                                            op0=ALU.add)
                    ip1s.append(ip1)
                Ef = Efp.tile([128, T], bf16, tag="Ef", name=f"Ef{h}")
                for half in range(2):
                    sl = slice(HALF * half, HALF * (half + 1))
                    z = gemm_bf16(0, h, half, f"zf{h}_{half}")
                    act_raw(Ef[:, sl], z[:], AF.Exp, bias=nbf_ap, scale=-1.0)
                    nc.vector.scalar_tensor_tensor(
                        out=p[:, sl], in0=Ef[:, sl], scalar=1.0,
                        in1=ip1s[half][:], op0=ALU.add, op1=ALU.mult,
                    )
                for half in range(2):
                    sl = slice(HALF * half, HALF * (half + 1))
                    zh = gemm_h(h, half, f"zh{h}_{half}")
                    th = shp.tile([128, HALF], bf16, tag="th",
                                  name=f"th{h}_{half}")
                    act_raw(th[:], zh[:], AF.Tanh, bias=bh2_ap, scale=0.5)
                    sh = shp.tile([128, HALF], bf16, tag="sh",
                                  name=f"sh{h}_{half}")
                    nc.vector.tensor_scalar(out=sh[:], in0=th[:],
                                            scalar1=0.5, scalar2=0.5,
                                            op0=ALU.mult, op1=ALU.add)
                    nc.vector.scalar_tensor_tensor(
                        out=g[:, sl], in0=zh[:], scalar=bg_ap, in1=sh[:],
                        op0=ALU.add, op1=ALU.max,
                    )
                u = up.tile([128, T], bf16, tag="u", name=f"u{h}")
                S = Sp.tile([128, T], bf16, tag="S", name=f"S{h}")
                QW = 1024
                if last:
                    # quarter-granular chain to shorten the kernel tail
                    for q in range(4):
                        q0 = QW * q
                        nc.vector.tensor_tensor(
                            out=u[:, q0:q0 + QW], in0=p[:, q0:q0 + QW],
                            in1=g[:, q0:q0 + QW], op=ALU.mult)
                        init = 1.0 if q == 0 else S[:, q0 - 1:q0]
                        nc.vector.tensor_tensor_scan(
                            S[:, q0 + 1:q0 + QW:2],
                            u[:, q0:q0 + QW:2],
                            u[:, q0 + 1:q0 + QW:2],
                            init, ALU.add, ALU.add)
                        if q == 0:
                            nc.vector.tensor_scalar(
                                out=S[:, 0:1], in0=u[:, 0:1], scalar1=1.0,
                                scalar2=None, op0=ALU.add)
                            nc.vector.tensor_tensor(
                                out=S[:, 2:QW:2], in0=S[:, 1:QW - 1:2],
                                in1=u[:, 2:QW:2], op=ALU.add)
                        else:
                            nc.vector.tensor_tensor(
                                out=S[:, q0:q0 + QW:2],
                                in0=S[:, q0 - 1:q0 + QW - 1:2],
                                in1=u[:, q0:q0 + QW:2], op=ALU.add)
                        if q % 2 == 1:
                            t0 = q0 + QW - HALF
                            nc.sync.dma_start(
                                S_d[128 * h:128 * (h + 1), t0:t0 + HALF],
                                S[:, t0:t0 + HALF])
                else:
                    for half in range(2):
                        sl = slice(HALF * half, HALF * (half + 1))
                        if U_ON_GP:
                            nc.gpsimd.tensor_tensor(
                                out=u[:, sl], in0=p[:, sl], in1=g[:, sl],
                                op=ALU.mult)
                        else:
                            nc.vector.tensor_tensor(
                                out=u[:, sl], in0=p[:, sl], in1=g[:, sl],
                                op=ALU.mult)
                        t0 = HALF * half
                        init = 1.0 if half == 0 else S[:, HALF - 1:HALF]
                        nc.vector.tensor_tensor_scan(
                            S[:, t0 + 1:t0 + HALF:2],
                            u[:, t0:t0 + HALF:2],
                            u[:, t0 + 1:t0 + HALF:2],
                            init, ALU.add, ALU.add)
                        if half == 0:
                            nc.vector.tensor_scalar(
                                out=S[:, 0:1], in0=u[:, 0:1], scalar1=1.0,
                                scalar2=None, op0=ALU.add)
                            nc.gpsimd.tensor_tensor(
                                out=S[:, 2:HALF:2], in0=S[:, 1:HALF - 1:2],
                                in1=u[:, 2:HALF:2], op=ALU.add)
                        else:
                            nc.gpsimd.tensor_tensor(
                                out=S[:, HALF:T:2], in0=S[:, HALF - 1:T - 1:2],
                                in1=u[:, HALF:T:2], op=ALU.add)
                        nc.sync.dma_start(
                            S_d[128 * h:128 * (h + 1), t0:t0 + HALF],
                            S[:, t0:t0 + HALF])
                return p

            def phase_R(hs, p_t):
                """r2~ = recip(2+p~) (recip table), DMA out.
                Host computes S~*r2~ = S*r2."""
                for h in hs:
                    r2 = rp.tile([128, T], bf16, tag="r2", name=f"r2_{h}")
                    act_raw(r2[:], p_t[h][:], AF.Reciprocal, bias=2.0)
                    nc.sync.dma_start(r2_d[128 * h:128 * (h + 1), :], r2[:])

            # tanh keeps Ef/ti/th on one ACT table; recip only for R
            p_t = {}
            p_t[0] = phase_IH(0)
            p_t[1] = phase_IH(1)
            phase_R([0, 1], p_t)
            p_t[2] = phase_IH(2)
            p_t[3] = phase_IH(3)
            phase_R([2, 3], p_t)
    return nc


def _get_nc():
    if "nc" not in _CACHE:
        nc = _build()
        if LDW_OPT:
            orig = nc.to_json_bytes
            nc.to_json_bytes = lambda: _dedup_ldweights(orig())
        _CACHE["nc"] = nc
    return _CACHE["nc"]


def _make_in_maps(x, Wf, bf, Wi, bi, Wh, bh):
    import ml_dtypes
    bft = ml_dtypes.bfloat16
    f8 = ml_dtypes.float8_e4m3fn

    x = np.asarray(x, dtype=np.float32)
    Wfi = np.concatenate([np.asarray(Wf), np.asarray(Wi)], axis=1)
    # wt: per 128-row d-block [Wf_d | Wi_d]
    wt = np.empty((D, 2 * H), dtype=bft)
    for d in range(N_D):
        blk = Wfi[128 * d:128 * (d + 1)]
        wt[128 * d:128 * (d + 1), 0:H] = blk[:, 0:H].astype(bft)
        wt[128 * d:128 * (d + 1), H:2 * H] = blk[:, H:2 * H].astype(bft)

    # fp8 h-gate: W*8, x/8 (net scale 1.0); DoubleRow kpair tiles
    Wh8 = (np.asarray(Wh, np.float32) * XSCALE).astype(f8)   # [512, 512]
    # w8 tile k: [128 p, 2 k, 512 h] with d = k*256 + kk*128 + p
    w8 = np.empty((2 * 128, 2 * H), dtype=f8)
    for k in range(2):
        for kk in range(2):
            rows = Wh8[k * 256 + kk * 128:k * 256 + kk * 128 + 128, :]
            w8[128 * k:128 * (k + 1), H * kk:H * (kk + 1)] = rows

    bf32 = np.asarray(bf, dtype=np.float32)
    bi32 = np.asarray(bi, dtype=np.float32)
    bh32 = np.asarray(bh, dtype=np.float32)
    biases = np.zeros((128, 16), dtype=np.float32)
    biases[:, 0:4] = (-bf32).reshape(N_HC, 128).T
    biases[:, 4:8] = (bi32 * np.float32(0.5)).reshape(N_HC, 128).T
    biases[:, 8:12] = (bh32 + np.float32(0.5)).reshape(N_HC, 128).T
    biases[:, 12:16] = (bh32 * np.float32(0.5)).reshape(N_HC, 128).T

    in_maps = []
    for c in range(NCORES):
        xT = np.ascontiguousarray(x[c].T)          # [512, 4096] f32
        xt16 = xT.astype(bft)
        x8T = (xT / XSCALE).astype(f8)
        x8 = np.empty((2 * 128, 2 * T), dtype=f8)
        for k in range(2):
            for kk in range(2):
                rows = x8T[k * 256 + kk * 128:k * 256 + kk * 128 + 128, :]
                x8[128 * k:128 * (k + 1), T * kk:T * (kk + 1)] = rows
        in_maps.append({"wt": wt, "xt": xt16, "x8": x8, "w8": w8,
                        "biases": biases})
    return in_maps


def kernel(x, Wf, bf, Wi, bi, Wh, bh):
    from concourse.bass_utils import run_bass_kernel_spmd

    in_maps = _make_in_maps(x, Wf, bf, Wi, bi, Wh, bh)
    nc = _get_nc()
    res = run_bass_kernel_spmd(nc, in_maps, list(range(NCORES)))

    out = np.empty((B, T + 1, H), dtype=np.float32)
    out[:, 0, :] = np.float32(0.5)
    for c in range(NCORES):
        S = np.asarray(res.results[c]["S"]).astype(np.float32)
        r2 = np.asarray(res.results[c]["r2"]).astype(np.float32)
        out[c, 1:, :] = (S * r2).T
    return out
